# revision 4
# baseline (speedup 1.0000x reference)
"""Single-head attention (B=8, S=2048, D=1024) on 8 TRN2 NeuronCores.

Data-parallel over batch: core b handles batch element b entirely.

Fast path (key compaction + fused QK):
  Keys with mask==0 contribute exactly zero after softmax, so the host
  compacts x^T to the unmasked key columns (padded to a multiple of 128;
  padded slots get bias -1e5 so exp underflows to 0). With ~50% masking
  this halves all key-side work.

  scores = (x Wq^T)(x_k Wk^T)^T = x M x_k^T with M = Wq^T Wk computed on
  host, eliminating the K projection entirely.

  All attention math runs in the transposed orientation scoresT[k, q]:
    - the mask/pad bias becomes per-partition, applied for free in the
      Exp activation (no mask matmuls),
    - exp(scoresT) chunks are directly the PV stationary (no PE
      transposes),
    - softmax denominators come from one tiny ones-vector matmul per
      128-query chunk accumulated in PSUM.

  Softmax shift-invariance: reference subtracts rowmax; we subtract
  nothing (scores are O(6), exp is safe in fp32) - identical result.
  Nonzero bq/bk fold into the per-key bias on host (the per-query term
  is softmax-invariant); bv is added on host (softmax rows sum to 1).

Fallback for sk_pad > 1280 (nearly unmasked inputs): the original
full-length kernel (no compaction), SBUF-safe at S=2048.
"""

import sys

sys.path.insert(0, "/opt/trn_rl_repo")

import numpy as np

import concourse.bacc as bacc
import concourse.tile as tile
from concourse import mybir
from concourse.bass_utils import run_bass_kernel_spmd

FP32R = mybir.dt.float32r
FP32 = mybir.dt.float32
BF16 = mybir.dt.bfloat16

S = 2048
D = 1024
NCORES = 8
NDC = D // 128      # 8 contraction chunks
QG = 512            # query group
NQG = S // QG       # 4
NQC = S // 128      # 16 (fallback kernel)
NKB = S // 512      # 4  (fallback kernel)
NEC = D // 128      # 8  (fallback kernel)
SCALE = 1.0 / np.sqrt(np.float32(D))
PAD_BIAS = -1.0e5


def build_nc_compact(sk_pad: int):
    nkc = sk_pad // 128
    nc = bacc.Bacc("TRN2", target_bir_lowering=False)

    XTK = nc.dram_tensor("XTK", [D, sk_pad], FP32R, kind="ExternalInput")
    XT = nc.dram_tensor("XT", [D, S], BF16, kind="ExternalInput")
    MQK = nc.dram_tensor("MQK", [D, D], BF16, kind="ExternalInput")
    WVT = nc.dram_tensor("WVT", [D, D], FP32R, kind="ExternalInput")
    MB = nc.dram_tensor("MB", [128, nkc], FP32, kind="ExternalInput")
    ONES = nc.dram_tensor("ONES", [128, 2], FP32R, kind="ExternalInput")
    OUT = nc.dram_tensor("OUT", [S, D], FP32, kind="ExternalOutput")

    Copy = mybir.ActivationFunctionType.Copy
    Exp = mybir.ActivationFunctionType.Exp

    with tile.TileContext(nc) as tc:
        with (
            tc.tile_pool(name="const", bufs=1) as constp,
            tc.tile_pool(name="res", bufs=1) as resp,
        ):
            ones = constp.tile([128, 2], FP32R)
            mb = constp.tile([128, nkc], FP32)
            nc.sync.dma_start(ones, ONES[:, :])
            nc.sync.dma_start(mb, MB[:, :])

            # residents: x_k^T [d%128][k], V [k%128, kc, e], M [d%128][d']
            # per-dc tiles so consumers wait per-slice; xtk/wvt DMAs
            # interleaved so the first V matmul starts ~3us in
            xtk = [resp.tile([128, sk_pad], FP32R, name=f"xtk{dc}")
                   for dc in range(NDC)]
            v = resp.tile([128, nkc, D], FP32R)
            m_sb = [resp.tile([128, D], BF16, name=f"msb{dc}")
                    for dc in range(NDC)]

            # ---- Phase A: V[k, e] = sum_d xtk[d, k] * WvT[d, e] ----
            with (
                tc.tile_pool(name="wvp", bufs=1) as wvp,
                tc.tile_pool(name="psV", bufs=2, space="PSUM") as psV,
            ):
                wvt = [wvp.tile([128, D], FP32R, name=f"wvt{dc}")
                       for dc in range(NDC)]
                for dc in range(NDC):
                    nc.sync.dma_start(xtk[dc], XTK[dc * 128:(dc + 1) * 128, :])
                    nc.sync.dma_start(wvt[dc], WVT[dc * 128:(dc + 1) * 128, :])
                for dc in range(NDC):
                    nc.sync.dma_start(m_sb[dc], MQK[dc * 128:(dc + 1) * 128, :])
                for kc in range(nkc):
                    ps_v = psV.tile([128, D], FP32, tag="psv")
                    for dc in range(NDC):
                        for eb in range(2):
                            nc.tensor.matmul(
                                ps_v[:, eb * 512:(eb + 1) * 512],
                                xtk[dc][:, kc * 128:(kc + 1) * 128],
                                wvt[dc][:, eb * 512:(eb + 1) * 512],
                                start=(dc == 0),
                                stop=(dc == NDC - 1),
                            )
                    nc.scalar.activation(out=v[:, kc, :], in_=ps_v, func=Copy)

            # ---- Phase B: per query group of 512 ----
            with (
                tc.tile_pool(name="xtp", bufs=2) as xtp,
                tc.tile_pool(name="xmtp", bufs=1) as xmtp,
                tc.tile_pool(name="esp", bufs=1) as esp,
                tc.tile_pool(name="smallp", bufs=4) as smallp,
                tc.tile_pool(name="outp", bufs=2) as outp,
                tc.tile_pool(name="psX", bufs=2, space="PSUM") as psX,
                tc.tile_pool(name="psO", bufs=2, space="PSUM") as psO,
                tc.tile_pool(name="psD", bufs=1, space="PSUM") as psD,
            ):
                for qg in range(NQG):
                    q0 = qg * QG
                    xt_qg = xtp.tile([128, NDC, QG], BF16, tag="xt")
                    nc.sync.dma_start(
                        xt_qg,
                        XT[:, q0:q0 + QG].rearrange("(c p) q -> p c q", p=128),
                    )

                    # XM^T[d', q] = sum_d M[d, d'] x^T[d, q]
                    xmt = xmtp.tile([128, NDC, QG], FP32R, tag="xmt")
                    for ec in range(NDC):
                        ps = psX.tile([128, QG], FP32, tag="psx")
                        for dc in range(NDC):
                            nc.tensor.matmul(
                                ps,
                                m_sb[dc][:, ec * 128:(ec + 1) * 128],
                                xt_qg[:, dc, :],
                                start=(dc == 0),
                                stop=(dc == NDC - 1),
                            )
                        nc.scalar.activation(out=xmt[:, ec, :], in_=ps, func=Copy)

                    # E^T[k, q] = exp(scale * sum_d xtk[d, k] xmt[d, q] + mb[k])
                    es = esp.tile([128, nkc, QG], FP32R, tag="es")
                    for kc in range(nkc):
                        ps = psX.tile([128, QG], FP32, tag="psx")
                        for dc in range(NDC):
                            nc.tensor.matmul(
                                ps,
                                xtk[dc][:, kc * 128:(kc + 1) * 128],
                                xmt[:, dc, :],
                                start=(dc == 0),
                                stop=(dc == NDC - 1),
                            )
                        nc.scalar.activation(
                            out=es[:, kc, :], in_=ps, func=Exp,
                            scale=float(SCALE), bias=mb[:, kc:kc + 1],
                        )

                    # PV + denominators; divide and store
                    psd = psD.tile([128, 2 * (QG // 128)], FP32, tag="psd")
                    for qc in range(QG // 128):
                        ps_o = psO.tile([128, D], FP32, tag="pso")
                        for kc in range(nkc):
                            for eb in range(2):
                                nc.tensor.matmul(
                                    ps_o[:, eb * 512:(eb + 1) * 512],
                                    es[:, kc, qc * 128:(qc + 1) * 128],
                                    v[:, kc, eb * 512:(eb + 1) * 512],
                                    start=(kc == 0),
                                    stop=(kc == nkc - 1),
                                )
                            nc.tensor.matmul(
                                psd[:, 2 * qc:2 * qc + 2],
                                es[:, kc, qc * 128:(qc + 1) * 128],
                                ones,
                                start=(kc == 0),
                                stop=(kc == nkc - 1),
                            )
                        recip = smallp.tile([128, 1], FP32, tag="recip")
                        nc.vector.reciprocal(recip, psd[:, 2 * qc:2 * qc + 1])
                        osb = outp.tile([128, D], FP32, tag="osb")
                        nc.vector.tensor_scalar_mul(osb, ps_o, recip)
                        nc.sync.dma_start(
                            OUT[q0 + qc * 128:q0 + (qc + 1) * 128, :], osb)

    nc.compile()
    return nc


def _compact_in_maps(x, mask, Wq, bq, Wk, bk, Wv, sk_pad):
    import ml_dtypes
    bf = ml_dtypes.bfloat16
    nkc = sk_pad // 128
    MQK = (Wq.T.astype(np.float64) @ Wk.astype(np.float64)).astype(bf)
    WVT = np.ascontiguousarray(Wv.T)
    ones = np.ones((128, 2), dtype=np.float32)
    bqWk = (bq.astype(np.float64) @ Wk.astype(np.float64)).astype(np.float64)

    in_maps = []
    for b in range(x.shape[0]):
        xT = np.ascontiguousarray(x[b].T)
        idx = np.flatnonzero(mask[b])
        cnt = len(idx)
        xtk = np.zeros((D, sk_pad), dtype=np.float32)
        xtk[:, :cnt] = xT[:, idx]
        xT_bf = xT.astype(bf)
        mb_full = np.full(sk_pad, PAD_BIAS, dtype=np.float32)
        if cnt:
            # per-key additive bias from bq (per-query term is
            # softmax-invariant and dropped)
            mb_full[:cnt] = (SCALE * (bqWk @ xtk[:, :cnt].astype(np.float64))
                             ).astype(np.float32)
        mb = np.ascontiguousarray(mb_full.reshape(nkc, 128).T)
        in_maps.append({
            "XTK": xtk, "XT": xT_bf, "MQK": MQK, "WVT": WVT,
            "MB": mb, "ONES": ones,
        })
    return in_maps


def _sk_pad_for(mask):
    cnts = (np.asarray(mask) != 0).sum(axis=1)
    return 128 * max(1, int(-(-int(cnts.max()) // 128)))


def _build_in_maps(inputs):
    """in_maps matching what kernel() runs for these inputs (test harness)."""
    x = np.asarray(inputs["x"], dtype=np.float32)
    mask = np.asarray(inputs["mask"])
    Wq = np.asarray(inputs["Wq"], dtype=np.float32)
    Wk = np.asarray(inputs["Wk"], dtype=np.float32)
    Wv = np.asarray(inputs["Wv"], dtype=np.float32)
    bq = np.asarray(inputs.get("bq", np.zeros(D)), dtype=np.float32)
    bk = np.asarray(inputs.get("bk", np.zeros(D)), dtype=np.float32)
    sk_pad = _sk_pad_for(mask)
    if sk_pad <= 1280:
        return _compact_in_maps(x, mask, Wq, bq, Wk, bk, Wv, sk_pad)
    return _full_in_maps(x, mask, Wq, bq, Wk, bk, Wv)


_NC_CACHE = {}


def kernel(x, mask, Wq, bq, Wk, bk, Wv, bv):
    x = np.asarray(x, dtype=np.float32)
    mask = np.asarray(mask)
    Wq = np.asarray(Wq, dtype=np.float32)
    Wk = np.asarray(Wk, dtype=np.float32)
    Wv = np.asarray(Wv, dtype=np.float32)
    bq = np.asarray(bq, dtype=np.float32)
    bk = np.asarray(bk, dtype=np.float32)
    bv = np.asarray(bv, dtype=np.float32)

    B = x.shape[0]
    assert x.shape == (B, S, D) and B == NCORES

    sk_pad = _sk_pad_for(mask)
    if sk_pad <= 1280:
        key = ("compact", sk_pad)
        if key not in _NC_CACHE:
            _NC_CACHE[key] = build_nc_compact(sk_pad)
        nc = _NC_CACHE[key]
        in_maps = _compact_in_maps(x, mask, Wq, bq, Wk, bk, Wv, sk_pad)
    else:
        has_bq = bool(np.any(bq != 0.0))
        has_bk = bool(np.any(bk != 0.0))
        key = ("full", has_bq, has_bk)
        if key not in _NC_CACHE:
            _NC_CACHE[key] = build_nc_full(has_bq, has_bk)
        nc = _NC_CACHE[key]
        in_maps = _full_in_maps(x, mask, Wq, bq, Wk, bk, Wv)

    res = run_bass_kernel_spmd(nc, in_maps, core_ids=list(range(NCORES)))
    out = np.stack([res.results[b]["OUT"] for b in range(B)], axis=0)
    if np.any(bv != 0.0):
        out = out + bv[None, None, :]

    # a batch with every key masked softmaxes uniformly over all keys
    cnts = (mask != 0).sum(axis=1)
    for b in range(B):
        if cnts[b] == 0:
            out[b, :, :] = x[b].mean(axis=0) @ Wv.T + bv
    return out.astype(np.float32)


# ---------------------------------------------------------------------------
# Fallback: original full-length kernel (no compaction), used only when
# nearly all keys are unmasked (sk_pad > 1280).
# ---------------------------------------------------------------------------


def build_nc_full(has_bq: bool, has_bk: bool):
    """Full-S kernel: K^T/V resident, Q^T staged via DRAM scratch,
    scores + softmax + PV per 128-query chunk with PE transposes."""
    nc = bacc.Bacc("TRN2", target_bir_lowering=False)

    XT = nc.dram_tensor("XT", [D, S], FP32R, kind="ExternalInput")
    WQT = nc.dram_tensor("WQT", [D, D], FP32R, kind="ExternalInput")
    WKT = nc.dram_tensor("WKT", [D, D], FP32R, kind="ExternalInput")
    WVT = nc.dram_tensor("WVT", [D, D], FP32R, kind="ExternalInput")
    MB = nc.dram_tensor("MB", [1, S], FP32R, kind="ExternalInput")
    ONES = nc.dram_tensor("ONES", [1, 128], FP32R, kind="ExternalInput")
    IDN = nc.dram_tensor("IDN", [128, 128], FP32R, kind="ExternalInput")
    BQ = nc.dram_tensor("BQ", [128, NEC], FP32, kind="ExternalInput")
    BK = nc.dram_tensor("BK", [128, NEC], FP32, kind="ExternalInput")
    OUT = nc.dram_tensor("OUT", [S, D], FP32, kind="ExternalOutput")

    Copy = mybir.ActivationFunctionType.Copy
    Exp = mybir.ActivationFunctionType.Exp

    with tile.TileContext(nc) as tc:
        with (
            tc.tile_pool(name="dram", bufs=1, space="DRAM") as dramp,
            tc.tile_pool(name="const", bufs=1) as constp,
            tc.tile_pool(name="ktp", bufs=1) as ktp,
        ):
            qt_dram = dramp.tile([NQC, NEC, 128, 128], FP32R)

            ident = constp.tile([128, 128], FP32R)
            ones = constp.tile([1, 128], FP32R)
            maskb = constp.tile([1, S], FP32R)
            nc.sync.dma_start(ident, IDN[:, :])
            nc.sync.dma_start(ones, ONES[:, :])
            nc.sync.dma_start(maskb, MB[:, :])
            bq_sb = constp.tile([128, NEC], FP32)
            bk_sb = constp.tile([128, NEC], FP32)
            if has_bq:
                nc.sync.dma_start(bq_sb, BQ[:, :])
            if has_bk:
                nc.sync.dma_start(bk_sb, BK[:, :])

            kt = ktp.tile([128, NEC, S], FP32R)  # K^T: [e%128, e//128, key]

            # ---- Phases 1+2: K^T and Q^T with x^T resident ----
            with (
                tc.tile_pool(name="xtp", bufs=1) as xtp,
                tc.tile_pool(name="wp", bufs=1) as wp,
                tc.tile_pool(name="qsp", bufs=2) as qsp,
                tc.tile_pool(name="psA", bufs=2, space="PSUM") as psA,
            ):
                xt = xtp.tile([128, NDC, S], FP32R)
                for d in range(NDC):
                    nc.sync.dma_start(
                        xt[:, d, :], XT[d * 128:(d + 1) * 128, :])

                for phase in ("k", "q"):
                    w_sb = wp.tile([128, NDC, D], FP32R, tag="w")
                    WT = WKT if phase == "k" else WQT
                    for d in range(NDC):
                        nc.sync.dma_start(
                            w_sb[:, d, :], WT[d * 128:(d + 1) * 128, :])
                    for ec in range(NEC):
                        ps_row = psA.tile([128, S], FP32, tag="psrow")
                        for d in range(NDC):
                            for kb in range(NKB):
                                nc.tensor.matmul(
                                    ps_row[:, kb * 512:(kb + 1) * 512],
                                    w_sb[:, d, ec * 128:(ec + 1) * 128],
                                    xt[:, d, kb * 512:(kb + 1) * 512],
                                    start=(d == 0),
                                    stop=(d == NDC - 1),
                                )
                        if phase == "k":
                            nc.scalar.activation(
                                out=kt[:, ec, :], in_=ps_row, func=Copy,
                                bias=bk_sb[:, ec:ec + 1] if has_bk else 0.0,
                            )
                        else:
                            qsb = qsp.tile([128, S], FP32R, tag="qsb")
                            nc.scalar.activation(
                                out=qsb, in_=ps_row, func=Copy,
                                bias=bq_sb[:, ec:ec + 1] if has_bq else 0.0,
                            )
                            nc.sync.dma_start(
                                qt_dram[:, ec].rearrange("qc p q -> p qc q"),
                                qsb.rearrange("p (qc q) -> p qc q", q=128),
                            )

            # ---- Phase 3: V with x^T streamed as stationary tiles ----
            with tc.tile_pool(name="vp", bufs=1) as vp:
                vq = [vp.tile([128, 4, D], FP32R, name=f"vq{i}", tag=f"vq{i}")
                      for i in range(4)]  # V quarters: [key%128, kc%4, e]
                with (
                    tc.tile_pool(name="wp2", bufs=1) as wp2,
                    tc.tile_pool(name="xsp", bufs=3) as xsp,
                    tc.tile_pool(name="psV", bufs=2, space="PSUM") as psV,
                ):
                    wv_sb = wp2.tile([128, NDC, D], FP32R)
                    for d in range(NDC):
                        nc.sync.dma_start(
                            wv_sb[:, d, :], WVT[d * 128:(d + 1) * 128, :])
                    for kc in range(NQC):
                        xts = xsp.tile([128, NDC, 128], FP32R, tag="xts")
                        nc.sync.dma_start(
                            xts,
                            XT[:, kc * 128:(kc + 1) * 128].rearrange(
                                "(c p) k -> p c k", p=128
                            ),
                        )
                        ps_v = psV.tile([128, D], FP32, tag="psv")
                        for d in range(NDC):
                            for eb in range(2):
                                nc.tensor.matmul(
                                    ps_v[:, eb * 512:(eb + 1) * 512],
                                    xts[:, d, :],
                                    wv_sb[:, d, eb * 512:(eb + 1) * 512],
                                    start=(d == 0),
                                    stop=(d == NDC - 1),
                                )
                        nc.scalar.activation(
                            out=vq[kc // 4][:, kc % 4, :], in_=ps_v, func=Copy)

                # ---- Phase B: attention per query chunk ----
                with (
                    tc.tile_pool(name="qtbp", bufs=3) as qtbp,
                    tc.tile_pool(name="esp", bufs=2) as esp,
                    tc.tile_pool(name="ptp", bufs=2) as ptp,
                    tc.tile_pool(name="outp", bufs=2) as outp,
                    tc.tile_pool(name="smallp", bufs=3) as smallp,
                    tc.tile_pool(name="psS", bufs=4, space="PSUM") as psS,
                    tc.tile_pool(name="psT", bufs=2, space="PSUM") as psT,
                    tc.tile_pool(name="psO", bufs=1, space="PSUM") as psO,
                ):
                    for qc in range(NQC):
                        qt_sb = qtbp.tile([128, NEC, 128], FP32R, tag="qt")
                        nc.sync.dma_start(
                            qt_sb, qt_dram[qc].rearrange("ec p q -> p ec q")
                        )

                        ps_s = [psS.tile([128, 512], FP32, tag="pss",
                                         name=f"pss_{qc}_{i}")
                                for i in range(NKB)]
                        for ec in range(NEC):
                            for kb in range(NKB):
                                nc.tensor.matmul(
                                    ps_s[kb],
                                    qt_sb[:, ec, :],
                                    kt[:, ec, kb * 512:(kb + 1) * 512],
                                    start=(ec == 0),
                                    stop=False,
                                )
                        for kb in range(NKB):
                            nc.tensor.matmul(
                                ps_s[kb],
                                ones,
                                maskb[:, kb * 512:(kb + 1) * 512],
                                start=False,
                                stop=True,
                            )

                        es = esp.tile([128, S], FP32R, tag="es")
                        pden = smallp.tile([128, NKB], FP32, tag="pden")
                        for kb in range(NKB):
                            nc.scalar.activation(
                                out=es[:, kb * 512:(kb + 1) * 512],
                                in_=ps_s[kb],
                                func=Exp,
                                scale=float(SCALE),
                                accum_out=pden[:, kb:kb + 1],
                            )
                        recip = smallp.tile([128, 1], FP32, tag="recip")
                        den = smallp.tile([128, 1], FP32, tag="den")
                        nc.vector.reduce_sum(den, pden, axis=mybir.AxisListType.X)
                        nc.vector.reciprocal(recip, den)

                        pt_g = [ptp.tile([128, 4, 128], FP32R,
                                         name=f"ptg{g}_{qc}", tag=f"ptg{g}")
                                for g in range(4)]
                        for g in range(4):
                            pt_ps = psT.tile([128, 4, 128], FP32R, tag="ptps")
                            for j in range(4):
                                kc = 4 * g + j
                                nc.tensor.transpose(
                                    pt_ps[:, j, :],
                                    es[:, kc * 128:(kc + 1) * 128], ident
                                )
                            nc.vector.tensor_copy(pt_g[g], pt_ps)

                        ps_o = psO.tile([128, D], FP32, tag="pso")
                        for kc in range(NQC):
                            for eb in range(2):
                                nc.tensor.matmul(
                                    ps_o[:, eb * 512:(eb + 1) * 512],
                                    pt_g[kc // 4][:, kc % 4, :],
                                    vq[kc // 4][:, kc % 4, eb * 512:(eb + 1) * 512],
                                    start=(kc == 0),
                                    stop=(kc == NQC - 1),
                                )
                        osb = outp.tile([128, D], FP32, tag="osb")
                        nc.vector.tensor_scalar_mul(osb, ps_o, recip)
                        nc.sync.dma_start(OUT[qc * 128:(qc + 1) * 128, :], osb)

    nc.compile()
    return nc


def _full_in_maps(x, mask, Wq, bq, Wk, bk, Wv):
    WQT = np.ascontiguousarray(Wq.T)
    WKT = np.ascontiguousarray(Wk.T)
    WVT = np.ascontiguousarray(Wv.T)
    ones = np.ones((1, 128), dtype=np.float32)
    idn = np.eye(128, dtype=np.float32)
    bq_r = np.ascontiguousarray(bq.reshape(NEC, 128).T)
    bk_r = np.ascontiguousarray(bk.reshape(NEC, 128).T)

    in_maps = []
    for b in range(x.shape[0]):
        xT = np.ascontiguousarray(x[b].T)
        mb = np.where(mask[b] != 0, 0.0, -1.0e6).astype(np.float32).reshape(1, S)
        in_maps.append({
            "XT": xT, "WQT": WQT, "WKT": WKT, "WVT": WVT,
            "MB": mb, "ONES": ones, "IDN": idn, "BQ": bq_r, "BK": bk_r,
        })
    return in_maps


# revision 5
# speedup vs baseline: 1.1359x; 1.1359x over previous
"""Single-head attention (B=8, S=2048, D=1024) on 8 TRN2 NeuronCores.

Data-parallel over batch: core b handles batch element b entirely.

Fast path (key compaction + fused QK):
  Keys with mask==0 contribute exactly zero after softmax, so the host
  compacts x^T to the unmasked key columns (padded to a multiple of 128;
  padded slots get bias -1e5 so exp underflows to 0). With ~50% masking
  this halves all key-side work.

  scores = (x Wq^T)(x_k Wk^T)^T = x M x_k^T with M = Wq^T Wk computed on
  host, eliminating the K projection entirely.

  All attention math runs in the transposed orientation scoresT[k, q]:
    - the mask/pad bias becomes per-partition, applied for free in the
      Exp activation (no mask matmuls),
    - exp(scoresT) chunks are directly the PV stationary (no PE
      transposes),
    - softmax denominators come from one tiny ones-vector matmul per
      128-query chunk accumulated in PSUM.

  Softmax shift-invariance: reference subtracts rowmax; we subtract
  nothing (scores are O(6), exp is safe in fp32) - identical result.
  Nonzero bq/bk fold into the per-key bias on host (the per-query term
  is softmax-invariant); bv is added on host (softmax rows sum to 1).

Fallback for sk_pad > 1280 (nearly unmasked inputs): the original
full-length kernel (no compaction), SBUF-safe at S=2048.
"""

import sys

sys.path.insert(0, "/opt/trn_rl_repo")

import numpy as np

import concourse.bacc as bacc
import concourse.tile as tile
from concourse import mybir
from concourse.bass_utils import run_bass_kernel_spmd

FP32R = mybir.dt.float32r
FP32 = mybir.dt.float32
BF16 = mybir.dt.bfloat16

S = 2048
D = 1024
NCORES = 8
NDC = D // 128      # 8 contraction chunks
QG = 512            # query group
NQG = S // QG       # 4
NQC = S // 128      # 16 (fallback kernel)
NKB = S // 512      # 4  (fallback kernel)
NEC = D // 128      # 8  (fallback kernel)
SCALE = 1.0 / np.sqrt(np.float32(D))
PAD_BIAS = -1.0e5


def build_nc_compact(sk_pad: int):
    nkc = sk_pad // 128
    nc = bacc.Bacc("TRN2", target_bir_lowering=False)

    XTK = nc.dram_tensor("XTK", [D, sk_pad], FP32R, kind="ExternalInput")
    XTR = nc.dram_tensor("XTR", [128, S * NDC], FP32R, kind="ExternalInput")
    MQK = nc.dram_tensor("MQK", [D, D], FP32R, kind="ExternalInput")
    WVT = nc.dram_tensor("WVT", [D, D], FP32R, kind="ExternalInput")
    MB = nc.dram_tensor("MB", [128, nkc], FP32, kind="ExternalInput")
    ONES = nc.dram_tensor("ONES", [128, 2], FP32R, kind="ExternalInput")
    OUT = nc.dram_tensor("OUT", [S, D], FP32, kind="ExternalOutput")

    Copy = mybir.ActivationFunctionType.Copy
    Exp = mybir.ActivationFunctionType.Exp

    with tile.TileContext(nc) as tc:
        with (
            tc.tile_pool(name="const", bufs=1) as constp,
            tc.tile_pool(name="res", bufs=1) as resp,
        ):
            ones = constp.tile([128, 2], FP32R)
            mb = constp.tile([128, nkc], FP32)
            nc.sync.dma_start(ones, ONES[:, :])
            nc.sync.dma_start(mb, MB[:, :])

            # residents: x_k^T [d%128][k], V [k%128, kc, e], M [d%128][d']
            # per-dc tiles so consumers wait per-slice; xtk/wvt DMAs
            # interleaved so the first V matmul starts ~3us in
            xtk = [resp.tile([128, sk_pad], FP32R, name=f"xtk{dc}")
                   for dc in range(NDC)]
            v = resp.tile([128, nkc, D], FP32R)
            m_sb = [resp.tile([128, D], FP32R, name=f"msb{dc}")
                    for dc in range(NDC)]

            # ---- Phase A: V[k, e] = sum_d xtk[d, k] * WvT[d, e] ----
            with (
                tc.tile_pool(name="wvp", bufs=1) as wvp,
                tc.tile_pool(name="psV", bufs=2, space="PSUM") as psV,
            ):
                wvt = [wvp.tile([128, D], FP32R, name=f"wvt{dc}")
                       for dc in range(NDC)]
                for dc in range(NDC):
                    nc.sync.dma_start(xtk[dc], XTK[dc * 128:(dc + 1) * 128, :])
                    nc.sync.dma_start(wvt[dc], WVT[dc * 128:(dc + 1) * 128, :])
                for dc in range(NDC):
                    nc.sync.dma_start(m_sb[dc], MQK[dc * 128:(dc + 1) * 128, :])
                for kc in range(nkc):
                    ps_v = psV.tile([128, D], FP32, tag="psv")
                    for dc in range(NDC):
                        for eb in range(2):
                            nc.tensor.matmul(
                                ps_v[:, eb * 512:(eb + 1) * 512],
                                xtk[dc][:, kc * 128:(kc + 1) * 128],
                                wvt[dc][:, eb * 512:(eb + 1) * 512],
                                start=(dc == 0),
                                stop=(dc == NDC - 1),
                            )
                    nc.scalar.activation(out=v[:, kc, :], in_=ps_v, func=Copy)

            # ---- Phase B: per query group of 512 ----
            with (
                tc.tile_pool(name="xtp", bufs=2) as xtp,
                tc.tile_pool(name="xmtp", bufs=1) as xmtp,
                tc.tile_pool(name="esp", bufs=1) as esp,
                tc.tile_pool(name="smallp", bufs=4) as smallp,
                tc.tile_pool(name="outp", bufs=2) as outp,
                tc.tile_pool(name="psX", bufs=2, space="PSUM") as psX,
                tc.tile_pool(name="psO", bufs=2, space="PSUM") as psO,
                tc.tile_pool(name="psD", bufs=1, space="PSUM") as psD,
            ):
                for qg in range(NQG):
                    q0 = qg * QG
                    xt_qg = xtp.tile([128, NDC, QG], FP32R, tag="xt")
                    nc.sync.dma_start(
                        xt_qg,
                        XTR[:, qg * NDC * QG:(qg + 1) * NDC * QG].rearrange(
                            "p (c q) -> p c q", q=QG),
                    )

                    # XM^T[d', q] = sum_d M[d, d'] x^T[d, q]
                    xmt = xmtp.tile([128, NDC, QG], FP32R, tag="xmt")
                    for ec in range(NDC):
                        ps = psX.tile([128, QG], FP32, tag="psx")
                        for dc in range(NDC):
                            nc.tensor.matmul(
                                ps,
                                m_sb[dc][:, ec * 128:(ec + 1) * 128],
                                xt_qg[:, dc, :],
                                start=(dc == 0),
                                stop=(dc == NDC - 1),
                            )
                        nc.scalar.activation(out=xmt[:, ec, :], in_=ps, func=Copy)

                    # E^T[k, q] = exp(scale * sum_d xtk[d, k] xmt[d, q] + mb[k])
                    es = esp.tile([128, nkc, QG], FP32R, tag="es")
                    for kc in range(nkc):
                        ps = psX.tile([128, QG], FP32, tag="psx")
                        for dc in range(NDC):
                            nc.tensor.matmul(
                                ps,
                                xtk[dc][:, kc * 128:(kc + 1) * 128],
                                xmt[:, dc, :],
                                start=(dc == 0),
                                stop=(dc == NDC - 1),
                            )
                        nc.scalar.activation(
                            out=es[:, kc, :], in_=ps, func=Exp,
                            scale=float(SCALE), bias=mb[:, kc:kc + 1],
                        )

                    # PV + denominators; divide and store
                    psd = psD.tile([128, 2 * (QG // 128)], FP32, tag="psd")
                    for qc in range(QG // 128):
                        ps_o = psO.tile([128, D], FP32, tag="pso")
                        for kc in range(nkc):
                            for eb in range(2):
                                nc.tensor.matmul(
                                    ps_o[:, eb * 512:(eb + 1) * 512],
                                    es[:, kc, qc * 128:(qc + 1) * 128],
                                    v[:, kc, eb * 512:(eb + 1) * 512],
                                    start=(kc == 0),
                                    stop=(kc == nkc - 1),
                                )
                            nc.tensor.matmul(
                                psd[:, 2 * qc:2 * qc + 2],
                                es[:, kc, qc * 128:(qc + 1) * 128],
                                ones,
                                start=(kc == 0),
                                stop=(kc == nkc - 1),
                            )
                        recip = smallp.tile([128, 1], FP32, tag="recip")
                        nc.vector.reciprocal(recip, psd[:, 2 * qc:2 * qc + 1])
                        osb = outp.tile([128, D], FP32, tag="osb")
                        nc.vector.tensor_scalar_mul(osb, ps_o, recip)
                        nc.sync.dma_start(
                            OUT[q0 + qc * 128:q0 + (qc + 1) * 128, :], osb)

    nc.compile()
    return nc


def _compact_in_maps(x, mask, Wq, bq, Wk, bk, Wv, sk_pad):
    nkc = sk_pad // 128
    MQK = (Wq.T.astype(np.float64) @ Wk.astype(np.float64)).astype(np.float32)
    WVT = np.ascontiguousarray(Wv.T)
    ones = np.ones((128, 2), dtype=np.float32)
    bqWk = (bq.astype(np.float64) @ Wk.astype(np.float64)).astype(np.float64)

    in_maps = []
    for b in range(x.shape[0]):
        xT = np.ascontiguousarray(x[b].T)
        idx = np.flatnonzero(mask[b])
        cnt = len(idx)
        xtk = np.zeros((D, sk_pad), dtype=np.float32)
        xtk[:, :cnt] = xT[:, idx]
        # [p, qg, dc, q'] = x[qg*512+q', dc*128+p], flattened per partition
        xtr = np.ascontiguousarray(
            x[b].reshape(NQG, QG, NDC, 128).transpose(3, 0, 2, 1)
        ).reshape(128, S * NDC)
        mb_full = np.full(sk_pad, PAD_BIAS, dtype=np.float32)
        if cnt:
            # per-key additive bias from bq (per-query term is
            # softmax-invariant and dropped)
            mb_full[:cnt] = (SCALE * (bqWk @ xtk[:, :cnt].astype(np.float64))
                             ).astype(np.float32)
        mb = np.ascontiguousarray(mb_full.reshape(nkc, 128).T)
        in_maps.append({
            "XTK": xtk, "XTR": xtr, "MQK": MQK, "WVT": WVT,
            "MB": mb, "ONES": ones,
        })
    return in_maps


def _sk_pad_for(mask):
    cnts = (np.asarray(mask) != 0).sum(axis=1)
    return 128 * max(1, int(-(-int(cnts.max()) // 128)))


def _build_in_maps(inputs):
    """in_maps matching what kernel() runs for these inputs (test harness)."""
    x = np.asarray(inputs["x"], dtype=np.float32)
    mask = np.asarray(inputs["mask"])
    Wq = np.asarray(inputs["Wq"], dtype=np.float32)
    Wk = np.asarray(inputs["Wk"], dtype=np.float32)
    Wv = np.asarray(inputs["Wv"], dtype=np.float32)
    bq = np.asarray(inputs.get("bq", np.zeros(D)), dtype=np.float32)
    bk = np.asarray(inputs.get("bk", np.zeros(D)), dtype=np.float32)
    sk_pad = _sk_pad_for(mask)
    if sk_pad <= 1280:
        return _compact_in_maps(x, mask, Wq, bq, Wk, bk, Wv, sk_pad)
    return _full_in_maps(x, mask, Wq, bq, Wk, bk, Wv)


_NC_CACHE = {}


def kernel(x, mask, Wq, bq, Wk, bk, Wv, bv):
    x = np.asarray(x, dtype=np.float32)
    mask = np.asarray(mask)
    Wq = np.asarray(Wq, dtype=np.float32)
    Wk = np.asarray(Wk, dtype=np.float32)
    Wv = np.asarray(Wv, dtype=np.float32)
    bq = np.asarray(bq, dtype=np.float32)
    bk = np.asarray(bk, dtype=np.float32)
    bv = np.asarray(bv, dtype=np.float32)

    B = x.shape[0]
    assert x.shape == (B, S, D) and B == NCORES

    sk_pad = _sk_pad_for(mask)
    if sk_pad <= 1280:
        key = ("compact", sk_pad)
        if key not in _NC_CACHE:
            _NC_CACHE[key] = build_nc_compact(sk_pad)
        nc = _NC_CACHE[key]
        in_maps = _compact_in_maps(x, mask, Wq, bq, Wk, bk, Wv, sk_pad)
    else:
        has_bq = bool(np.any(bq != 0.0))
        has_bk = bool(np.any(bk != 0.0))
        key = ("full", has_bq, has_bk)
        if key not in _NC_CACHE:
            _NC_CACHE[key] = build_nc_full(has_bq, has_bk)
        nc = _NC_CACHE[key]
        in_maps = _full_in_maps(x, mask, Wq, bq, Wk, bk, Wv)

    res = run_bass_kernel_spmd(nc, in_maps, core_ids=list(range(NCORES)))
    out = np.stack([res.results[b]["OUT"] for b in range(B)], axis=0)
    if np.any(bv != 0.0):
        out = out + bv[None, None, :]

    # a batch with every key masked softmaxes uniformly over all keys
    cnts = (mask != 0).sum(axis=1)
    for b in range(B):
        if cnts[b] == 0:
            out[b, :, :] = x[b].mean(axis=0) @ Wv.T + bv
    return out.astype(np.float32)


# ---------------------------------------------------------------------------
# Fallback: original full-length kernel (no compaction), used only when
# nearly all keys are unmasked (sk_pad > 1280).
# ---------------------------------------------------------------------------


def build_nc_full(has_bq: bool, has_bk: bool):
    """Full-S kernel: K^T/V resident, Q^T staged via DRAM scratch,
    scores + softmax + PV per 128-query chunk with PE transposes."""
    nc = bacc.Bacc("TRN2", target_bir_lowering=False)

    XT = nc.dram_tensor("XT", [D, S], FP32R, kind="ExternalInput")
    WQT = nc.dram_tensor("WQT", [D, D], FP32R, kind="ExternalInput")
    WKT = nc.dram_tensor("WKT", [D, D], FP32R, kind="ExternalInput")
    WVT = nc.dram_tensor("WVT", [D, D], FP32R, kind="ExternalInput")
    MB = nc.dram_tensor("MB", [1, S], FP32R, kind="ExternalInput")
    ONES = nc.dram_tensor("ONES", [1, 128], FP32R, kind="ExternalInput")
    IDN = nc.dram_tensor("IDN", [128, 128], FP32R, kind="ExternalInput")
    BQ = nc.dram_tensor("BQ", [128, NEC], FP32, kind="ExternalInput")
    BK = nc.dram_tensor("BK", [128, NEC], FP32, kind="ExternalInput")
    OUT = nc.dram_tensor("OUT", [S, D], FP32, kind="ExternalOutput")

    Copy = mybir.ActivationFunctionType.Copy
    Exp = mybir.ActivationFunctionType.Exp

    with tile.TileContext(nc) as tc:
        with (
            tc.tile_pool(name="dram", bufs=1, space="DRAM") as dramp,
            tc.tile_pool(name="const", bufs=1) as constp,
            tc.tile_pool(name="ktp", bufs=1) as ktp,
        ):
            qt_dram = dramp.tile([NQC, NEC, 128, 128], FP32R)

            ident = constp.tile([128, 128], FP32R)
            ones = constp.tile([1, 128], FP32R)
            maskb = constp.tile([1, S], FP32R)
            nc.sync.dma_start(ident, IDN[:, :])
            nc.sync.dma_start(ones, ONES[:, :])
            nc.sync.dma_start(maskb, MB[:, :])
            bq_sb = constp.tile([128, NEC], FP32)
            bk_sb = constp.tile([128, NEC], FP32)
            if has_bq:
                nc.sync.dma_start(bq_sb, BQ[:, :])
            if has_bk:
                nc.sync.dma_start(bk_sb, BK[:, :])

            kt = ktp.tile([128, NEC, S], FP32R)  # K^T: [e%128, e//128, key]

            # ---- Phases 1+2: K^T and Q^T with x^T resident ----
            with (
                tc.tile_pool(name="xtp", bufs=1) as xtp,
                tc.tile_pool(name="wp", bufs=1) as wp,
                tc.tile_pool(name="qsp", bufs=2) as qsp,
                tc.tile_pool(name="psA", bufs=2, space="PSUM") as psA,
            ):
                xt = xtp.tile([128, NDC, S], FP32R)
                for d in range(NDC):
                    nc.sync.dma_start(
                        xt[:, d, :], XT[d * 128:(d + 1) * 128, :])

                for phase in ("k", "q"):
                    w_sb = wp.tile([128, NDC, D], FP32R, tag="w")
                    WT = WKT if phase == "k" else WQT
                    for d in range(NDC):
                        nc.sync.dma_start(
                            w_sb[:, d, :], WT[d * 128:(d + 1) * 128, :])
                    for ec in range(NEC):
                        ps_row = psA.tile([128, S], FP32, tag="psrow")
                        for d in range(NDC):
                            for kb in range(NKB):
                                nc.tensor.matmul(
                                    ps_row[:, kb * 512:(kb + 1) * 512],
                                    w_sb[:, d, ec * 128:(ec + 1) * 128],
                                    xt[:, d, kb * 512:(kb + 1) * 512],
                                    start=(d == 0),
                                    stop=(d == NDC - 1),
                                )
                        if phase == "k":
                            nc.scalar.activation(
                                out=kt[:, ec, :], in_=ps_row, func=Copy,
                                bias=bk_sb[:, ec:ec + 1] if has_bk else 0.0,
                            )
                        else:
                            qsb = qsp.tile([128, S], FP32R, tag="qsb")
                            nc.scalar.activation(
                                out=qsb, in_=ps_row, func=Copy,
                                bias=bq_sb[:, ec:ec + 1] if has_bq else 0.0,
                            )
                            nc.sync.dma_start(
                                qt_dram[:, ec].rearrange("qc p q -> p qc q"),
                                qsb.rearrange("p (qc q) -> p qc q", q=128),
                            )

            # ---- Phase 3: V with x^T streamed as stationary tiles ----
            with tc.tile_pool(name="vp", bufs=1) as vp:
                vq = [vp.tile([128, 4, D], FP32R, name=f"vq{i}", tag=f"vq{i}")
                      for i in range(4)]  # V quarters: [key%128, kc%4, e]
                with (
                    tc.tile_pool(name="wp2", bufs=1) as wp2,
                    tc.tile_pool(name="xsp", bufs=3) as xsp,
                    tc.tile_pool(name="psV", bufs=2, space="PSUM") as psV,
                ):
                    wv_sb = wp2.tile([128, NDC, D], FP32R)
                    for d in range(NDC):
                        nc.sync.dma_start(
                            wv_sb[:, d, :], WVT[d * 128:(d + 1) * 128, :])
                    for kc in range(NQC):
                        xts = xsp.tile([128, NDC, 128], FP32R, tag="xts")
                        nc.sync.dma_start(
                            xts,
                            XT[:, kc * 128:(kc + 1) * 128].rearrange(
                                "(c p) k -> p c k", p=128
                            ),
                        )
                        ps_v = psV.tile([128, D], FP32, tag="psv")
                        for d in range(NDC):
                            for eb in range(2):
                                nc.tensor.matmul(
                                    ps_v[:, eb * 512:(eb + 1) * 512],
                                    xts[:, d, :],
                                    wv_sb[:, d, eb * 512:(eb + 1) * 512],
                                    start=(d == 0),
                                    stop=(d == NDC - 1),
                                )
                        nc.scalar.activation(
                            out=vq[kc // 4][:, kc % 4, :], in_=ps_v, func=Copy)

                # ---- Phase B: attention per query chunk ----
                with (
                    tc.tile_pool(name="qtbp", bufs=3) as qtbp,
                    tc.tile_pool(name="esp", bufs=2) as esp,
                    tc.tile_pool(name="ptp", bufs=2) as ptp,
                    tc.tile_pool(name="outp", bufs=2) as outp,
                    tc.tile_pool(name="smallp", bufs=3) as smallp,
                    tc.tile_pool(name="psS", bufs=4, space="PSUM") as psS,
                    tc.tile_pool(name="psT", bufs=2, space="PSUM") as psT,
                    tc.tile_pool(name="psO", bufs=1, space="PSUM") as psO,
                ):
                    for qc in range(NQC):
                        qt_sb = qtbp.tile([128, NEC, 128], FP32R, tag="qt")
                        nc.sync.dma_start(
                            qt_sb, qt_dram[qc].rearrange("ec p q -> p ec q")
                        )

                        ps_s = [psS.tile([128, 512], FP32, tag="pss",
                                         name=f"pss_{qc}_{i}")
                                for i in range(NKB)]
                        for ec in range(NEC):
                            for kb in range(NKB):
                                nc.tensor.matmul(
                                    ps_s[kb],
                                    qt_sb[:, ec, :],
                                    kt[:, ec, kb * 512:(kb + 1) * 512],
                                    start=(ec == 0),
                                    stop=False,
                                )
                        for kb in range(NKB):
                            nc.tensor.matmul(
                                ps_s[kb],
                                ones,
                                maskb[:, kb * 512:(kb + 1) * 512],
                                start=False,
                                stop=True,
                            )

                        es = esp.tile([128, S], FP32R, tag="es")
                        pden = smallp.tile([128, NKB], FP32, tag="pden")
                        for kb in range(NKB):
                            nc.scalar.activation(
                                out=es[:, kb * 512:(kb + 1) * 512],
                                in_=ps_s[kb],
                                func=Exp,
                                scale=float(SCALE),
                                accum_out=pden[:, kb:kb + 1],
                            )
                        recip = smallp.tile([128, 1], FP32, tag="recip")
                        den = smallp.tile([128, 1], FP32, tag="den")
                        nc.vector.reduce_sum(den, pden, axis=mybir.AxisListType.X)
                        nc.vector.reciprocal(recip, den)

                        pt_g = [ptp.tile([128, 4, 128], FP32R,
                                         name=f"ptg{g}_{qc}", tag=f"ptg{g}")
                                for g in range(4)]
                        for g in range(4):
                            pt_ps = psT.tile([128, 4, 128], FP32R, tag="ptps")
                            for j in range(4):
                                kc = 4 * g + j
                                nc.tensor.transpose(
                                    pt_ps[:, j, :],
                                    es[:, kc * 128:(kc + 1) * 128], ident
                                )
                            nc.vector.tensor_copy(pt_g[g], pt_ps)

                        ps_o = psO.tile([128, D], FP32, tag="pso")
                        for kc in range(NQC):
                            for eb in range(2):
                                nc.tensor.matmul(
                                    ps_o[:, eb * 512:(eb + 1) * 512],
                                    pt_g[kc // 4][:, kc % 4, :],
                                    vq[kc // 4][:, kc % 4, eb * 512:(eb + 1) * 512],
                                    start=(kc == 0),
                                    stop=(kc == NQC - 1),
                                )
                        osb = outp.tile([128, D], FP32, tag="osb")
                        nc.vector.tensor_scalar_mul(osb, ps_o, recip)
                        nc.sync.dma_start(OUT[qc * 128:(qc + 1) * 128, :], osb)

    nc.compile()
    return nc


def _full_in_maps(x, mask, Wq, bq, Wk, bk, Wv):
    WQT = np.ascontiguousarray(Wq.T)
    WKT = np.ascontiguousarray(Wk.T)
    WVT = np.ascontiguousarray(Wv.T)
    ones = np.ones((1, 128), dtype=np.float32)
    idn = np.eye(128, dtype=np.float32)
    bq_r = np.ascontiguousarray(bq.reshape(NEC, 128).T)
    bk_r = np.ascontiguousarray(bk.reshape(NEC, 128).T)

    in_maps = []
    for b in range(x.shape[0]):
        xT = np.ascontiguousarray(x[b].T)
        mb = np.where(mask[b] != 0, 0.0, -1.0e6).astype(np.float32).reshape(1, S)
        in_maps.append({
            "XT": xT, "WQT": WQT, "WKT": WKT, "WVT": WVT,
            "MB": mb, "ONES": ones, "IDN": idn, "BQ": bq_r, "BK": bk_r,
        })
    return in_maps


# revision 6
# speedup vs baseline: 1.2082x; 1.0637x over previous
"""Single-head attention (B=8, S=2048, D=1024) on 8 TRN2 NeuronCores.

Data-parallel over batch: core b handles batch element b entirely.

Fast path (key compaction + fused QK):
  Keys with mask==0 contribute exactly zero after softmax, so the host
  compacts x^T to the unmasked key columns (padded to a multiple of 128;
  padded slots get bias -1e5 so exp underflows to 0). With ~50% masking
  this halves all key-side work.

  scores = (x Wq^T)(x_k Wk^T)^T = x M x_k^T with M = Wq^T Wk computed on
  host, eliminating the K projection entirely.

  All attention math runs in the transposed orientation scoresT[k, q]:
    - the mask/pad bias becomes per-partition, applied for free in the
      Exp activation (no mask matmuls),
    - exp(scoresT) chunks are directly the PV stationary (no PE
      transposes),
    - softmax denominators come from one tiny ones-vector matmul per
      128-query chunk accumulated in PSUM.

  Softmax shift-invariance: reference subtracts rowmax; we subtract
  nothing (scores are O(6), exp is safe in fp32) - identical result.
  Nonzero bq/bk fold into the per-key bias on host (the per-query term
  is softmax-invariant); bv is added on host (softmax rows sum to 1).

Fallback for sk_pad > 1280 (nearly unmasked inputs): the original
full-length kernel (no compaction), SBUF-safe at S=2048.
"""

import sys

sys.path.insert(0, "/opt/trn_rl_repo")

import numpy as np

import concourse.bacc as bacc
import concourse.tile as tile
from concourse import mybir
from concourse.bass_utils import run_bass_kernel_spmd

FP32R = mybir.dt.float32r
FP32 = mybir.dt.float32
BF16 = mybir.dt.bfloat16

S = 2048
D = 1024
NCORES = 8
NDC = D // 128      # 8 contraction chunks
QG = 512            # query group
NQG = S // QG       # 4
NQC = S // 128      # 16 (fallback kernel)
NKB = S // 512      # 4  (fallback kernel)
NEC = D // 128      # 8  (fallback kernel)
SCALE = 1.0 / np.sqrt(np.float32(D))
PAD_BIAS = -1.0e5


def build_nc_compact(sk_pad: int):
    nkc = sk_pad // 128
    nc = bacc.Bacc("TRN2", target_bir_lowering=False)

    XTK = nc.dram_tensor("XTK", [D, sk_pad], FP32R, kind="ExternalInput")
    XTR = nc.dram_tensor("XTR", [128, S * NDC], FP32R, kind="ExternalInput")
    MQK = nc.dram_tensor("MQK", [D, D], FP32R, kind="ExternalInput")
    WVT = nc.dram_tensor("WVT", [D, D], FP32R, kind="ExternalInput")
    MB = nc.dram_tensor("MB", [128, nkc], FP32, kind="ExternalInput")
    ONES = nc.dram_tensor("ONES", [128, 2], FP32R, kind="ExternalInput")
    OUT = nc.dram_tensor("OUT", [S, D], FP32, kind="ExternalOutput")

    Copy = mybir.ActivationFunctionType.Copy
    Exp = mybir.ActivationFunctionType.Exp

    with tile.TileContext(nc) as tc:
        with (
            tc.tile_pool(name="const", bufs=1) as constp,
            tc.tile_pool(name="res", bufs=1) as resp,
        ):
            ones = constp.tile([128, 2], FP32R)
            mb = constp.tile([128, nkc], FP32)

            # residents: x_k^T [d%128][k], V [k%128, kc, e], M [d%128][d']
            # per-dc tiles so consumers wait per-slice; xtk/wvt DMAs
            # interleaved so the first V matmul starts ~3us in
            xtk = [resp.tile([128, sk_pad], FP32R, name=f"xtk{dc}")
                   for dc in range(NDC)]
            v = resp.tile([128, nkc, D], FP32R)
            m_sb = resp.tile([128, NDC, D], FP32R)

            # ---- Phase A: V[k, e] = sum_d xtk[d, k] * WvT[d, e] ----
            with (
                tc.tile_pool(name="wvp", bufs=1) as wvp,
                tc.tile_pool(name="psV", bufs=2, space="PSUM") as psV,
            ):
                wvt = wvp.tile([128, NDC, D], FP32R)
                nc.sync.dma_start(xtk[0], XTK[0:128, :])
                nc.sync.dma_start(
                    wvt, WVT.rearrange("(c p) e -> p c e", p=128))
                for dc in range(1, NDC):
                    nc.sync.dma_start(xtk[dc], XTK[dc * 128:(dc + 1) * 128, :])
                nc.sync.dma_start(
                    m_sb, MQK.rearrange("(c p) e -> p c e", p=128))
                nc.sync.dma_start(mb, MB[:, :])
                nc.sync.dma_start(ones, ONES[:, :])
                for kc in range(nkc):
                    ps_v = psV.tile([128, D], FP32, tag="psv")
                    for dc in range(NDC):
                        for eb in range(2):
                            nc.tensor.matmul(
                                ps_v[:, eb * 512:(eb + 1) * 512],
                                xtk[dc][:, kc * 128:(kc + 1) * 128],
                                wvt[:, dc, eb * 512:(eb + 1) * 512],
                                start=(dc == 0),
                                stop=(dc == NDC - 1),
                            )
                    nc.scalar.activation(out=v[:, kc, :], in_=ps_v, func=Copy)

            # ---- Phase B: per query group of 512 ----
            with (
                tc.tile_pool(name="xtp", bufs=2) as xtp,
                tc.tile_pool(name="xmtp", bufs=1) as xmtp,
                tc.tile_pool(name="esp", bufs=1) as esp,
                tc.tile_pool(name="denp", bufs=1) as denp,
                tc.tile_pool(name="smallp", bufs=4) as smallp,
                tc.tile_pool(name="outp", bufs=2) as outp,
                tc.tile_pool(name="psX", bufs=2, space="PSUM") as psX,
                tc.tile_pool(name="psO", bufs=2, space="PSUM") as psO,
                tc.tile_pool(name="psD", bufs=1, space="PSUM") as psD,
            ):
                for qg in range(NQG):
                    q0 = qg * QG
                    xt_qg = xtp.tile([128, NDC, QG], FP32R, tag="xt")
                    nc.sync.dma_start(
                        xt_qg,
                        XTR[:, qg * NDC * QG:(qg + 1) * NDC * QG].rearrange(
                            "p (c q) -> p c q", q=QG),
                    )

                    # XM^T[d', q] = sum_d M[d, d'] x^T[d, q]
                    xmt = xmtp.tile([128, NDC, QG], FP32R, tag="xmt")
                    for ec in range(NDC):
                        ps = psX.tile([128, QG], FP32, tag="psx")
                        for dc in range(NDC):
                            nc.tensor.matmul(
                                ps,
                                m_sb[:, dc, ec * 128:(ec + 1) * 128],
                                xt_qg[:, dc, :],
                                start=(dc == 0),
                                stop=(dc == NDC - 1),
                            )
                        nc.scalar.activation(out=xmt[:, ec, :], in_=ps, func=Copy)

                    # E^T[k, q] = exp(scale * sum_d xtk[d, k] xmt[d, q] + mb[k])
                    es = esp.tile([128, nkc, QG], FP32R, tag="es")
                    den = denp.tile([128, QG], FP32R, tag="den")
                    for kc in range(nkc):
                        ps = psX.tile([128, QG], FP32, tag="psx")
                        for dc in range(NDC):
                            nc.tensor.matmul(
                                ps,
                                xtk[dc][:, kc * 128:(kc + 1) * 128],
                                xmt[:, dc, :],
                                start=(dc == 0),
                                stop=(dc == NDC - 1),
                            )
                        nc.scalar.activation(
                            out=es[:, kc, :], in_=ps, func=Exp,
                            scale=float(SCALE), bias=mb[:, kc:kc + 1],
                        )
                        if kc == 0:
                            nc.vector.tensor_copy(den, es[:, kc, :])
                        else:
                            nc.vector.tensor_add(den, den, es[:, kc, :])

                    # PV + denominators; divide and store
                    psd = psD.tile([128, 2 * (QG // 128)], FP32, tag="psd")
                    for qc in range(QG // 128):
                        ps_o = psO.tile([128, D], FP32, tag="pso")
                        for kc in range(nkc):
                            for eb in range(2):
                                nc.tensor.matmul(
                                    ps_o[:, eb * 512:(eb + 1) * 512],
                                    es[:, kc, qc * 128:(qc + 1) * 128],
                                    v[:, kc, eb * 512:(eb + 1) * 512],
                                    start=(kc == 0),
                                    stop=(kc == nkc - 1),
                                )
                        nc.tensor.matmul(
                            psd[:, 2 * qc:2 * qc + 2],
                            den[:, qc * 128:(qc + 1) * 128],
                            ones,
                            start=True,
                            stop=True,
                        )
                        recip = smallp.tile([128, 1], FP32, tag="recip")
                        nc.vector.reciprocal(recip, psd[:, 2 * qc:2 * qc + 1])
                        osb = outp.tile([128, D], FP32, tag="osb")
                        nc.vector.tensor_scalar_mul(osb, ps_o, recip)
                        nc.sync.dma_start(
                            OUT[q0 + qc * 128:q0 + (qc + 1) * 128, :], osb)

    nc.compile()
    return nc


def _compact_in_maps(x, mask, Wq, bq, Wk, bk, Wv, sk_pad):
    nkc = sk_pad // 128
    MQK = (Wq.T.astype(np.float64) @ Wk.astype(np.float64)).astype(np.float32)
    WVT = np.ascontiguousarray(Wv.T)
    ones = np.ones((128, 2), dtype=np.float32)
    bqWk = (bq.astype(np.float64) @ Wk.astype(np.float64)).astype(np.float64)

    in_maps = []
    for b in range(x.shape[0]):
        xT = np.ascontiguousarray(x[b].T)
        idx = np.flatnonzero(mask[b])
        cnt = len(idx)
        xtk = np.zeros((D, sk_pad), dtype=np.float32)
        xtk[:, :cnt] = xT[:, idx]
        # [p, qg, dc, q'] = x[qg*512+q', dc*128+p], flattened per partition
        xtr = np.ascontiguousarray(
            x[b].reshape(NQG, QG, NDC, 128).transpose(3, 0, 2, 1)
        ).reshape(128, S * NDC)
        mb_full = np.full(sk_pad, PAD_BIAS, dtype=np.float32)
        if cnt:
            # per-key additive bias from bq (per-query term is
            # softmax-invariant and dropped)
            mb_full[:cnt] = (SCALE * (bqWk @ xtk[:, :cnt].astype(np.float64))
                             ).astype(np.float32)
        mb = np.ascontiguousarray(mb_full.reshape(nkc, 128).T)
        in_maps.append({
            "XTK": xtk, "XTR": xtr, "MQK": MQK, "WVT": WVT,
            "MB": mb, "ONES": ones,
        })
    return in_maps


def _sk_pad_for(mask):
    cnts = (np.asarray(mask) != 0).sum(axis=1)
    return 128 * max(1, int(-(-int(cnts.max()) // 128)))


def _build_in_maps(inputs):
    """in_maps matching what kernel() runs for these inputs (test harness)."""
    x = np.asarray(inputs["x"], dtype=np.float32)
    mask = np.asarray(inputs["mask"])
    Wq = np.asarray(inputs["Wq"], dtype=np.float32)
    Wk = np.asarray(inputs["Wk"], dtype=np.float32)
    Wv = np.asarray(inputs["Wv"], dtype=np.float32)
    bq = np.asarray(inputs.get("bq", np.zeros(D)), dtype=np.float32)
    bk = np.asarray(inputs.get("bk", np.zeros(D)), dtype=np.float32)
    sk_pad = _sk_pad_for(mask)
    if sk_pad <= 1280:
        return _compact_in_maps(x, mask, Wq, bq, Wk, bk, Wv, sk_pad)
    return _full_in_maps(x, mask, Wq, bq, Wk, bk, Wv)


_NC_CACHE = {}


def kernel(x, mask, Wq, bq, Wk, bk, Wv, bv):
    x = np.asarray(x, dtype=np.float32)
    mask = np.asarray(mask)
    Wq = np.asarray(Wq, dtype=np.float32)
    Wk = np.asarray(Wk, dtype=np.float32)
    Wv = np.asarray(Wv, dtype=np.float32)
    bq = np.asarray(bq, dtype=np.float32)
    bk = np.asarray(bk, dtype=np.float32)
    bv = np.asarray(bv, dtype=np.float32)

    B = x.shape[0]
    assert x.shape == (B, S, D) and B == NCORES

    sk_pad = _sk_pad_for(mask)
    if sk_pad <= 1280:
        key = ("compact", sk_pad)
        if key not in _NC_CACHE:
            _NC_CACHE[key] = build_nc_compact(sk_pad)
        nc = _NC_CACHE[key]
        in_maps = _compact_in_maps(x, mask, Wq, bq, Wk, bk, Wv, sk_pad)
    else:
        has_bq = bool(np.any(bq != 0.0))
        has_bk = bool(np.any(bk != 0.0))
        key = ("full", has_bq, has_bk)
        if key not in _NC_CACHE:
            _NC_CACHE[key] = build_nc_full(has_bq, has_bk)
        nc = _NC_CACHE[key]
        in_maps = _full_in_maps(x, mask, Wq, bq, Wk, bk, Wv)

    res = run_bass_kernel_spmd(nc, in_maps, core_ids=list(range(NCORES)))
    out = np.stack([res.results[b]["OUT"] for b in range(B)], axis=0)
    if np.any(bv != 0.0):
        out = out + bv[None, None, :]

    # a batch with every key masked softmaxes uniformly over all keys
    cnts = (mask != 0).sum(axis=1)
    for b in range(B):
        if cnts[b] == 0:
            out[b, :, :] = x[b].mean(axis=0) @ Wv.T + bv
    return out.astype(np.float32)


# ---------------------------------------------------------------------------
# Fallback: original full-length kernel (no compaction), used only when
# nearly all keys are unmasked (sk_pad > 1280).
# ---------------------------------------------------------------------------


def build_nc_full(has_bq: bool, has_bk: bool):
    """Full-S kernel: K^T/V resident, Q^T staged via DRAM scratch,
    scores + softmax + PV per 128-query chunk with PE transposes."""
    nc = bacc.Bacc("TRN2", target_bir_lowering=False)

    XT = nc.dram_tensor("XT", [D, S], FP32R, kind="ExternalInput")
    WQT = nc.dram_tensor("WQT", [D, D], FP32R, kind="ExternalInput")
    WKT = nc.dram_tensor("WKT", [D, D], FP32R, kind="ExternalInput")
    WVT = nc.dram_tensor("WVT", [D, D], FP32R, kind="ExternalInput")
    MB = nc.dram_tensor("MB", [1, S], FP32R, kind="ExternalInput")
    ONES = nc.dram_tensor("ONES", [1, 128], FP32R, kind="ExternalInput")
    IDN = nc.dram_tensor("IDN", [128, 128], FP32R, kind="ExternalInput")
    BQ = nc.dram_tensor("BQ", [128, NEC], FP32, kind="ExternalInput")
    BK = nc.dram_tensor("BK", [128, NEC], FP32, kind="ExternalInput")
    OUT = nc.dram_tensor("OUT", [S, D], FP32, kind="ExternalOutput")

    Copy = mybir.ActivationFunctionType.Copy
    Exp = mybir.ActivationFunctionType.Exp

    with tile.TileContext(nc) as tc:
        with (
            tc.tile_pool(name="dram", bufs=1, space="DRAM") as dramp,
            tc.tile_pool(name="const", bufs=1) as constp,
            tc.tile_pool(name="ktp", bufs=1) as ktp,
        ):
            qt_dram = dramp.tile([NQC, NEC, 128, 128], FP32R)

            ident = constp.tile([128, 128], FP32R)
            ones = constp.tile([1, 128], FP32R)
            maskb = constp.tile([1, S], FP32R)
            nc.sync.dma_start(ident, IDN[:, :])
            nc.sync.dma_start(ones, ONES[:, :])
            nc.sync.dma_start(maskb, MB[:, :])
            bq_sb = constp.tile([128, NEC], FP32)
            bk_sb = constp.tile([128, NEC], FP32)
            if has_bq:
                nc.sync.dma_start(bq_sb, BQ[:, :])
            if has_bk:
                nc.sync.dma_start(bk_sb, BK[:, :])

            kt = ktp.tile([128, NEC, S], FP32R)  # K^T: [e%128, e//128, key]

            # ---- Phases 1+2: K^T and Q^T with x^T resident ----
            with (
                tc.tile_pool(name="xtp", bufs=1) as xtp,
                tc.tile_pool(name="wp", bufs=1) as wp,
                tc.tile_pool(name="qsp", bufs=2) as qsp,
                tc.tile_pool(name="psA", bufs=2, space="PSUM") as psA,
            ):
                xt = xtp.tile([128, NDC, S], FP32R)
                for d in range(NDC):
                    nc.sync.dma_start(
                        xt[:, d, :], XT[d * 128:(d + 1) * 128, :])

                for phase in ("k", "q"):
                    w_sb = wp.tile([128, NDC, D], FP32R, tag="w")
                    WT = WKT if phase == "k" else WQT
                    for d in range(NDC):
                        nc.sync.dma_start(
                            w_sb[:, d, :], WT[d * 128:(d + 1) * 128, :])
                    for ec in range(NEC):
                        ps_row = psA.tile([128, S], FP32, tag="psrow")
                        for d in range(NDC):
                            for kb in range(NKB):
                                nc.tensor.matmul(
                                    ps_row[:, kb * 512:(kb + 1) * 512],
                                    w_sb[:, d, ec * 128:(ec + 1) * 128],
                                    xt[:, d, kb * 512:(kb + 1) * 512],
                                    start=(d == 0),
                                    stop=(d == NDC - 1),
                                )
                        if phase == "k":
                            nc.scalar.activation(
                                out=kt[:, ec, :], in_=ps_row, func=Copy,
                                bias=bk_sb[:, ec:ec + 1] if has_bk else 0.0,
                            )
                        else:
                            qsb = qsp.tile([128, S], FP32R, tag="qsb")
                            nc.scalar.activation(
                                out=qsb, in_=ps_row, func=Copy,
                                bias=bq_sb[:, ec:ec + 1] if has_bq else 0.0,
                            )
                            nc.sync.dma_start(
                                qt_dram[:, ec].rearrange("qc p q -> p qc q"),
                                qsb.rearrange("p (qc q) -> p qc q", q=128),
                            )

            # ---- Phase 3: V with x^T streamed as stationary tiles ----
            with tc.tile_pool(name="vp", bufs=1) as vp:
                vq = [vp.tile([128, 4, D], FP32R, name=f"vq{i}", tag=f"vq{i}")
                      for i in range(4)]  # V quarters: [key%128, kc%4, e]
                with (
                    tc.tile_pool(name="wp2", bufs=1) as wp2,
                    tc.tile_pool(name="xsp", bufs=3) as xsp,
                    tc.tile_pool(name="psV", bufs=2, space="PSUM") as psV,
                ):
                    wv_sb = wp2.tile([128, NDC, D], FP32R)
                    for d in range(NDC):
                        nc.sync.dma_start(
                            wv_sb[:, d, :], WVT[d * 128:(d + 1) * 128, :])
                    for kc in range(NQC):
                        xts = xsp.tile([128, NDC, 128], FP32R, tag="xts")
                        nc.sync.dma_start(
                            xts,
                            XT[:, kc * 128:(kc + 1) * 128].rearrange(
                                "(c p) k -> p c k", p=128
                            ),
                        )
                        ps_v = psV.tile([128, D], FP32, tag="psv")
                        for d in range(NDC):
                            for eb in range(2):
                                nc.tensor.matmul(
                                    ps_v[:, eb * 512:(eb + 1) * 512],
                                    xts[:, d, :],
                                    wv_sb[:, d, eb * 512:(eb + 1) * 512],
                                    start=(d == 0),
                                    stop=(d == NDC - 1),
                                )
                        nc.scalar.activation(
                            out=vq[kc // 4][:, kc % 4, :], in_=ps_v, func=Copy)

                # ---- Phase B: attention per query chunk ----
                with (
                    tc.tile_pool(name="qtbp", bufs=3) as qtbp,
                    tc.tile_pool(name="esp", bufs=2) as esp,
                    tc.tile_pool(name="ptp", bufs=2) as ptp,
                    tc.tile_pool(name="outp", bufs=2) as outp,
                    tc.tile_pool(name="smallp", bufs=3) as smallp,
                    tc.tile_pool(name="psS", bufs=4, space="PSUM") as psS,
                    tc.tile_pool(name="psT", bufs=2, space="PSUM") as psT,
                    tc.tile_pool(name="psO", bufs=1, space="PSUM") as psO,
                ):
                    for qc in range(NQC):
                        qt_sb = qtbp.tile([128, NEC, 128], FP32R, tag="qt")
                        nc.sync.dma_start(
                            qt_sb, qt_dram[qc].rearrange("ec p q -> p ec q")
                        )

                        ps_s = [psS.tile([128, 512], FP32, tag="pss",
                                         name=f"pss_{qc}_{i}")
                                for i in range(NKB)]
                        for ec in range(NEC):
                            for kb in range(NKB):
                                nc.tensor.matmul(
                                    ps_s[kb],
                                    qt_sb[:, ec, :],
                                    kt[:, ec, kb * 512:(kb + 1) * 512],
                                    start=(ec == 0),
                                    stop=False,
                                )
                        for kb in range(NKB):
                            nc.tensor.matmul(
                                ps_s[kb],
                                ones,
                                maskb[:, kb * 512:(kb + 1) * 512],
                                start=False,
                                stop=True,
                            )

                        es = esp.tile([128, S], FP32R, tag="es")
                        pden = smallp.tile([128, NKB], FP32, tag="pden")
                        for kb in range(NKB):
                            nc.scalar.activation(
                                out=es[:, kb * 512:(kb + 1) * 512],
                                in_=ps_s[kb],
                                func=Exp,
                                scale=float(SCALE),
                                accum_out=pden[:, kb:kb + 1],
                            )
                        recip = smallp.tile([128, 1], FP32, tag="recip")
                        den = smallp.tile([128, 1], FP32, tag="den")
                        nc.vector.reduce_sum(den, pden, axis=mybir.AxisListType.X)
                        nc.vector.reciprocal(recip, den)

                        pt_g = [ptp.tile([128, 4, 128], FP32R,
                                         name=f"ptg{g}_{qc}", tag=f"ptg{g}")
                                for g in range(4)]
                        for g in range(4):
                            pt_ps = psT.tile([128, 4, 128], FP32R, tag="ptps")
                            for j in range(4):
                                kc = 4 * g + j
                                nc.tensor.transpose(
                                    pt_ps[:, j, :],
                                    es[:, kc * 128:(kc + 1) * 128], ident
                                )
                            nc.vector.tensor_copy(pt_g[g], pt_ps)

                        ps_o = psO.tile([128, D], FP32, tag="pso")
                        for kc in range(NQC):
                            for eb in range(2):
                                nc.tensor.matmul(
                                    ps_o[:, eb * 512:(eb + 1) * 512],
                                    pt_g[kc // 4][:, kc % 4, :],
                                    vq[kc // 4][:, kc % 4, eb * 512:(eb + 1) * 512],
                                    start=(kc == 0),
                                    stop=(kc == NQC - 1),
                                )
                        osb = outp.tile([128, D], FP32, tag="osb")
                        nc.vector.tensor_scalar_mul(osb, ps_o, recip)
                        nc.sync.dma_start(OUT[qc * 128:(qc + 1) * 128, :], osb)

    nc.compile()
    return nc


def _full_in_maps(x, mask, Wq, bq, Wk, bk, Wv):
    WQT = np.ascontiguousarray(Wq.T)
    WKT = np.ascontiguousarray(Wk.T)
    WVT = np.ascontiguousarray(Wv.T)
    ones = np.ones((1, 128), dtype=np.float32)
    idn = np.eye(128, dtype=np.float32)
    bq_r = np.ascontiguousarray(bq.reshape(NEC, 128).T)
    bk_r = np.ascontiguousarray(bk.reshape(NEC, 128).T)

    in_maps = []
    for b in range(x.shape[0]):
        xT = np.ascontiguousarray(x[b].T)
        mb = np.where(mask[b] != 0, 0.0, -1.0e6).astype(np.float32).reshape(1, S)
        in_maps.append({
            "XT": xT, "WQT": WQT, "WKT": WKT, "WVT": WVT,
            "MB": mb, "ONES": ones, "IDN": idn, "BQ": bq_r, "BK": bk_r,
        })
    return in_maps
